# revision 8
# baseline (speedup 1.0000x reference)
"""Binarized 3-layer MLP on 8 TRN2 NeuronCores (data-parallel over batch).

Computation (matching the reference):
    h1  = x @ sign(W1).T          x: [65536, 784] fp32, W1: [400, 784]
    h2  = sign(h1) @ sign(W2).T   W2: [200, 400]
    out = sign(h2) @ sign(W3).T   W3: [10, 200]

Strategy (v2 — fp8 DoubleRow):
  - Batch sharded 8192 rows/core; weights replicated. Activations feature-major
    (features on SBUF partitions) so every contraction is already on partitions.
  - Layer 1 precision: x = hi + lo with hi = fp16(x), lo = fp16(x - hi) (exact).
    hi matmuls run in fp16 (K=784). The lo correction runs as fp8 DoubleRow:
    lo is quantized to e4m3 scaled by 2^12 and the weights carry sign(W1)*2^-12
    in e5m2 (exactly representable); one DR matmul contracts K=256. Total
    sign-flip error vs the fp32 reference measures rel=0.0071 on the actual
    inputs (gate is 2e-2) — dominated by the e4m3's 4-bit mantissa on lo,
    i.e. ~15 significand bits on x.
  - Layers 2/3 operate on exact +-1 values: e4m3 holds them exactly and fp32
    PSUM accumulation is exact, so layer 2 runs as fp8 DoubleRow (2 matmuls
    of K=256 instead of 4 of K=128) and layer 3 as plain fp8.
  - Layer-2 K layout: DR pairs are (partition p, half i). K-tile0 pairs
    h1 features (p | 128+p) = (m0 | m1) sign outputs; K-tile1 pairs
    (256+p | m4-packed strip). The m4 strip tile has sign outputs only at
    partitions 32jj:32jj+16 (chunk jj of the 4-chunk group, matching the
    col-strip-packed layer-1 m4 PSUM); weights for the other partitions are
    zero, and sign(memset-0 PSUM) = 0, so both operands vanish there.
  - The 400-row layer-1 output tiles as 128+128+128+16. The 16-row remainder
    (m4) is packed into one PSUM bank at partition strips 0/32/64/96 via
    tile_position col-tiling (4 chunks' matmuls run concurrently in distinct
    32-col PE groups). memset-to-zero + start=False keeps interleaved strip
    accumulation correct. Layer 3 (M=10) packs the same way.
  - K remainders (rows 768:784 of hi and lo) are folded into one 32-row fp16
    matmul per m-tile (lo is exact in fp16), replicated at partition strips
    0/32/64 so the three m-tiles' tail matmuls run concurrently.
"""

import contextlib
import ctypes
import os
import sys
import types

import numpy as np
import ml_dtypes

import concourse.bacc as bacc
import concourse.mybir as mybir
import concourse.tile as tile
from concourse.bass_utils import run_bass_kernel_spmd


def _ensure_axon_hooks():
    """concourse's trace path imports antenv.axon_hooks, which this image
    lacks; register a ctypes-backed stand-in so trace=True (or a stray
    BASS_TRACE=1 in the environment) cannot crash the run."""
    try:
        import antenv.axon_hooks  # noqa: F401
        return
    except ImportError:
        pass

    so_path = "/opt/axon/libaxon_pjrt.so"
    hook = None
    if os.path.exists(so_path):
        try:
            lib = ctypes.CDLL(so_path)
            if hasattr(lib, "axon_start_nrt_profile"):
                lib.axon_start_nrt_profile.argtypes = [
                    ctypes.POINTER(ctypes.c_int64),
                    ctypes.c_size_t,
                ]
                lib.axon_start_nrt_profile.restype = ctypes.c_int64
                lib.axon_stop_nrt_profile.argtypes = [ctypes.c_char_p]
                lib.axon_stop_nrt_profile.restype = ctypes.c_int64

                @contextlib.contextmanager
                def _hook(output_dir, device_ids):
                    import jax

                    jax.devices()
                    if device_ids:
                        ids = (ctypes.c_int64 * len(device_ids))(*device_ids)
                        rc = lib.axon_start_nrt_profile(ids, len(device_ids))
                    else:
                        rc = lib.axon_start_nrt_profile(None, 0)
                    if rc != 0:
                        raise RuntimeError(f"axon_start_nrt_profile rc={rc}")
                    try:
                        yield
                    finally:
                        lib.axon_stop_nrt_profile(str(output_dir).encode())

                hook = _hook
        except OSError:
            pass

    mod = types.ModuleType("antenv.axon_hooks")
    mod.get_axon_ntff_profile_hook = lambda: hook
    mod.set_axon_ntff_profile_hook = lambda h: None
    sys.modules["antenv.axon_hooks"] = mod

    import concourse.bass_utils as _bu

    _bu.upload_artifacts = lambda tmpdir: tmpdir

BF16 = np.dtype(ml_dtypes.bfloat16)
E4 = np.dtype(ml_dtypes.float8_e4m3)
E5 = np.dtype(ml_dtypes.float8_e5m2)

NCORES = 8
B = 65536
BL = B // NCORES          # 8192 rows per core
D0, H1, H2, DO = 784, 400, 200, 10
CH = 512                  # batch columns per chunk (PSUM bank = 512 fp32)
NCH = BL // CH            # 16 chunks per core
GRP = 4                   # chunks per packing group
KHI = 6                   # full 128-row fp16 k-tiles (rows 0:768)
KLO = 3                   # fp8 DoubleRow k-tiles of 256 (rows 0:768)
LSC = 2.0 ** 12           # lo scale: rhs carries lo*2^12, weights sign*2^-12
H2P = 208                 # padded layer-2 M so DR weight pair-stride % 16 == 0

_cache = {}


def _build():
    if "nc" in _cache:
        return _cache["nc"]

    f32 = mybir.dt.float32
    f16 = mybir.dt.float16
    f8e4 = mybir.dt.float8e4
    f8e5 = mybir.dt.float8e5
    Sign = mybir.ActivationFunctionType.Sign
    DR = mybir.MatmulPerfMode.DoubleRow

    nc = bacc.Bacc("TRN2", debug=False, num_devices=NCORES)

    d_xhi = nc.dram_tensor("xhi", [NCH, 128, KHI, CH], f16, kind="ExternalInput").ap()
    d_xlo = nc.dram_tensor("xlo", [NCH, 128, KLO, 2, CH], f8e4, kind="ExternalInput").ap()
    d_xtl = nc.dram_tensor("xtl", [NCH, 96, CH], f16, kind="ExternalInput").ap()
    # w1hi split so the first m-slab lands before the rest
    d_w1ha = nc.dram_tensor("w1ha", [128, KHI, 128], f16, kind="ExternalInput").ap()
    d_w1hb = nc.dram_tensor("w1hb", [128, KHI, H1 - 128], f16, kind="ExternalInput").ap()
    d_w1lo = nc.dram_tensor("w1lo", [128, KLO, 2, H1], f8e5, kind="ExternalInput").ap()
    d_w1tl = nc.dram_tensor("w1tl", [96, H1], f16, kind="ExternalInput").ap()
    d_w2a = nc.dram_tensor("w2a", [128, 2, H2P], f8e4, kind="ExternalInput").ap()
    d_w2b = nc.dram_tensor("w2b", [128, GRP, 2, H2P], f8e4, kind="ExternalInput").ap()
    d_w3 = nc.dram_tensor("w3", [128, 2, DO], f8e4, kind="ExternalInput").ap()
    d_out = nc.dram_tensor("out", [NCH, DO, CH], f32, kind="ExternalOutput").ap()

    with tile.TileContext(nc) as tc:
        with (
            tc.tile_pool(name="wp", bufs=1) as wp,
            tc.tile_pool(name="xp", bufs=6) as xp,
            tc.tile_pool(name="ap_", bufs=2) as apool,
            tc.tile_pool(name="a2p", bufs=2) as a2pool,
            tc.tile_pool(name="op", bufs=2) as op,
            tc.tile_pool(name="ps1p", bufs=1, space="PSUM") as ps1p,
            tc.tile_pool(name="ps2p", bufs=1, space="PSUM") as ps2p,
            tc.tile_pool(name="pspk", bufs=2, space="PSUM") as pspk,
        ):
            w1ha = wp.tile([128, KHI, 128], f16, name="w1ha")
            w1hb = wp.tile([128, KHI, H1 - 128], f16, name="w1hb")
            w1lo = wp.tile([128, KLO, 2, H1], f8e5, name="w1lo")
            w1tl = wp.tile([96, H1], f16, name="w1tl")
            w2a = wp.tile([128, 2, H2P], f8e4, name="w2a")
            w2b = wp.tile([128, GRP, 2, H2P], f8e4, name="w2b")
            w3sb = wp.tile([128, 2, DO], f8e4, name="w3sb")
            nc.sync.dma_start(out=w1ha[:], in_=d_w1ha)

            def w1h_slice(k, m_off, m_sz):
                if m_off == 0:
                    return w1ha[:, k, 0:m_sz]
                return w1hb[:, k, m_off - 128 : m_off - 128 + m_sz]

            def layer1_m123(xhi, xlo, xtl, pending=()):
                """Full-width layer-1 m-tiles; returns the chunk's a1 tile
                [128, 4, CH] e4m3 with halves (m0 | m1 | m2 | m4-packed);
                the m4 half is written separately from ps4.

                DoubleRow matmuls pay ~190ns extra when issued back-to-back
                (their 256-col weight loads cannot prefetch behind another
                DR), so the 3 DR matmuls of each m-tile are interleaved
                between fp16 matmuls, and `pending` (the previous chunk's
                layer-2 DR closures) is drained into fp16-separated slots."""
                pending = list(pending)
                a1 = apool.tile([128, 4, CH], f8e4, name="a1")
                pss = []
                for m in range(3):
                    ps = ps1p.tile(
                        [128, CH], f32, name=f"ps1_{m}", bufs=(2 if m == 0 else 1)
                    )

                    def hi(k, m=m, ps=ps):
                        nc.tensor.matmul(
                            ps[:],
                            w1h_slice(k, m * 128, 128),
                            xhi[:, k, :],
                            start=(k == 0),
                            stop=False,
                        )

                    def lo(t, m=m, ps=ps):
                        nc.tensor.matmul(
                            ps[:],
                            w1lo[:, t, :, m * 128 : m * 128 + 128],
                            xlo[:, t, :, :],
                            start=False,
                            stop=False,
                            perf_mode=DR,
                        )

                    # F P F d F d F P F d F  (P = pending slot, d = own DR)
                    hi(0)
                    if pending:
                        pending.pop(0)()
                    hi(1)
                    lo(0)
                    hi(2)
                    lo(1)
                    hi(3)
                    if pending:
                        pending.pop(0)()
                    hi(4)
                    lo(2)
                    hi(5)
                    pss.append(ps)
                # 32-row K tails (hi rows 768:784 + lo rows 768:784 as fp16),
                # replicated at partition strips 0/32/64 -> concurrent
                for m in range(3):
                    s = 32 * m
                    nc.tensor.matmul(
                        pss[m][:],
                        w1tl[s : s + 32, m * 128 : m * 128 + 128],
                        xtl[s : s + 32, :],
                        start=False,
                        stop=True,
                        tile_position=(s, 0),
                    )
                for p in pending:
                    p()
                for m in range(3):
                    nc.scalar.activation(a1[:, m, :], pss[m][:], Sign)
                return a1

            def layer2_make(jj, a1, a2s):
                """Returns 4 emit-closures (the two DR matmuls per m-tile);
                closures allocate PSUM/output tiles at emission time and
                record the sign outputs into a2s[jj]."""
                cells = {}

                def k0(m):
                    sz = 128 if m == 0 else 72
                    ps = ps2p.tile([sz, CH], f32, name=f"ps2_{m}")
                    cells[m] = ps
                    nc.tensor.matmul(
                        ps[:],
                        w2a[:, :, m * 128 : m * 128 + sz],
                        a1[:, 0:2, :],
                        start=True,
                        stop=False,
                        perf_mode=DR,
                    )

                def k1(m):
                    sz = 128 if m == 0 else 72
                    ps = cells[m]
                    nc.tensor.matmul(
                        ps[:],
                        w2b[:, jj, :, m * 128 : m * 128 + sz],
                        a1[:, 2:4, :],
                        start=False,
                        stop=True,
                        perf_mode=DR,
                    )
                    at = a2pool.tile([sz, CH], f8e4, name=f"a2_{jj}_{m}")
                    nc.scalar.activation(at[:], ps[:], Sign)
                    a2s[jj][m] = at

                return [
                    lambda: k0(0),
                    lambda: k1(0),
                    lambda: k0(1),
                    lambda: k1(1),
                ]

            # HAM/P-state pre-warm: dummy matmuls on a scratch tile keep the
            # PE busy during the initial weight/x DMA wait so the first real
            # matmuls run at full clock (the activity window is ~3.4us).
            warm = wp.tile([128, 64], f16, name="warm")
            nc.vector.memset(warm[:], 1.0)
            wps = pspk.tile([64, 64], f32, name="wps", tag="pack")
            for _ in range(48):
                nc.tensor.matmul(wps[:], warm[:, 0:64], warm[:], start=True, stop=True)

            for g in range(NCH // GRP):
                xhis, xlos, xtls = [], [], []
                for jj in range(GRP):
                    c = g * GRP + jj
                    xhi = xp.tile([128, KHI, CH], f16, name="xhi")
                    xlo = xp.tile([128, KLO, 2, CH], f8e4, name="xlo")
                    xtl = xp.tile([96, CH], f16, name="xtl")
                    nc.sync.dma_start(out=xhi[:], in_=d_xhi[c])
                    nc.sync.dma_start(out=xlo[:], in_=d_xlo[c])
                    nc.sync.dma_start(out=xtl[:], in_=d_xtl[c])
                    xhis.append(xhi)
                    xlos.append(xlo)
                    xtls.append(xtl)
                    if g == 0 and jj == 0:
                        nc.sync.dma_start(out=w1hb[:], in_=d_w1hb)
                        nc.sync.dma_start(out=w1lo[:], in_=d_w1lo)
                        nc.sync.dma_start(out=w1tl[:], in_=d_w1tl)
                    if g == 0 and jj == 1:
                        nc.sync.dma_start(out=w2a[:], in_=d_w2a)
                        nc.sync.dma_start(out=w2b[:], in_=d_w2b)
                        nc.sync.dma_start(out=w3sb[:], in_=d_w3)

                # packed m4 PSUM bank: strips [32jj : 32jj+16] per chunk
                ps4 = pspk.tile([128, CH], f32, name="ps4", tag="pack")
                nc.vector.memset(ps4[:], 0.0)

                a1s = [None] * GRP
                a1s[0] = layer1_m123(xhis[0], xlos[0], xtls[0])
                a1s[1] = layer1_m123(xhis[1], xlos[1], xtls[1])

                # m4 packed: 4 col-tiled strips, interleaved for concurrency
                for k in range(KHI):
                    for jj in range(GRP):
                        s = 32 * jj
                        nc.tensor.matmul(
                            ps4[s : s + 16, :],
                            w1h_slice(k, 384, 16),
                            xhis[jj][:, k, :],
                            start=False,
                            stop=False,
                            tile_position=(0, s),
                        )
                for t in range(KLO):
                    for i in range(2):
                        for jj in range(GRP):
                            s = 32 * jj
                            nc.tensor.matmul(
                                ps4[s : s + 16, :],
                                w1lo[:, t, i, 384:400],
                                xlos[jj][:, t, i, :],
                                start=False,
                                stop=False,
                                tile_position=(0, s),
                            )
                for jj in range(GRP):
                    s = 32 * jj
                    nc.tensor.matmul(
                        ps4[s : s + 16, :],
                        w1tl[0:32, 384:400],
                        xtls[jj][0:32, :],
                        start=False,
                        stop=True,
                        tile_position=(0, s),
                    )
                # m4 sign into each chunk's a1[:, 3, :] (zeros elsewhere:
                # sign(0) = 0, and the paired w2b weights are 0 there too)
                nc.scalar.activation(a1s[0][:, 3, :], ps4[:], Sign)
                nc.scalar.activation(a1s[1][:, 3, :], ps4[:], Sign)

                # layer 3, packed into one PSUM bank at strips [32jj:32jj+10];
                # its plain-fp8 matmuls double as DR separators below
                a2s = [[None, None] for _ in range(GRP)]
                ps3 = pspk.tile([128, CH], f32, name="ps3", tag="pack")
                nc.vector.memset(ps3[:], 0.0)

                def l3(jj, k):
                    ks = 128 if k == 0 else 72
                    s = 32 * jj
                    nc.tensor.matmul(
                        ps3[s : s + DO, :],
                        w3sb[0:ks, k, :],
                        a2s[jj][k][0:ks, :],
                        start=False,
                        stop=(k == 1),
                        tile_position=(0, s),
                    )

                # layer-2 DR closures drain into the next chunk's fp16 stream
                l2_0 = layer2_make(0, a1s[0], a2s)
                l2_1 = layer2_make(1, a1s[1], a2s)
                a1s[2] = layer1_m123(xhis[2], xlos[2], xtls[2], pending=l2_0)
                nc.scalar.activation(a1s[2][:, 3, :], ps4[:], Sign)
                l2_2 = layer2_make(2, a1s[2], a2s)
                a1s[3] = layer1_m123(xhis[3], xlos[3], xtls[3], pending=l2_1)
                nc.scalar.activation(a1s[3][:, 3, :], ps4[:], Sign)
                l2_3 = layer2_make(3, a1s[3], a2s)

                # remaining layer-2 DRs, separated by layer-3 plain matmuls
                # (each strip's k=0 matmul is emitted before its k=1)
                l2_2[0]()
                l3(0, 0)
                l2_2[1]()
                l3(1, 0)
                l2_2[2]()
                l3(0, 1)
                l2_2[3]()
                l3(1, 1)
                l2_3[0]()
                l3(2, 0)
                l2_3[1]()
                l3(2, 1)
                l2_3[2]()
                l2_3[3]()
                l3(3, 0)
                l3(3, 1)
                osb = op.tile([128, CH], f32, name="osb")
                nc.vector.tensor_copy(osb[:], ps3[:])
                for jj in range(GRP):
                    s = 32 * jj
                    nc.sync.dma_start(
                        out=d_out[g * GRP + jj], in_=osb[s : s + DO, :]
                    )

    nc.compile()
    _cache["nc"] = nc
    return nc


def _prep_weights(W1, W2, W3):
    s1T = np.sign(W1).T.astype(np.float32)  # [784, 400]
    # hi weights: rows 0:768 as 6 k-tiles of 128
    w1h = np.ascontiguousarray(
        s1T[:768].reshape(KHI, 128, H1).transpose(1, 0, 2)
    ).astype(np.float16)  # [128, 6, 400]
    w1ha = np.ascontiguousarray(w1h[:, :, 0:128])
    w1hb = np.ascontiguousarray(w1h[:, :, 128:H1])
    # lo weights: rows 0:768 as 3 DR k-tiles of (2 x 128), scaled 2^-12 (e5m2)
    w1lo = np.ascontiguousarray(
        (s1T[:768] / LSC).reshape(KLO, 2, 128, H1).transpose(2, 0, 1, 3)
    ).astype(E5)  # [128, 3, 2, 400]
    # K tail (rows 768:784): strips 0/32/64, each [hi-tail | lo-tail] with
    # identical +-1 weights (the rhs carries hi and lo values separately)
    w1tl = np.zeros((96, H1), np.float32)
    for s in (0, 32, 64):
        w1tl[s : s + 16] = s1T[768:784]
        w1tl[s + 16 : s + 32] = s1T[768:784]
    w1tl = w1tl.astype(np.float16)

    s2T = np.sign(W2).T.astype(np.float32)  # [400, 200]
    w2a = np.zeros((128, 2, H2P), np.float32)
    w2a[:, 0, :H2] = s2T[0:128]
    w2a[:, 1, :H2] = s2T[128:256]
    w2a = w2a.astype(E4)
    w2b = np.zeros((128, GRP, 2, H2P), np.float32)
    for jj in range(GRP):
        w2b[:, jj, 0, :H2] = s2T[256:384]
        w2b[32 * jj : 32 * jj + 16, jj, 1, :H2] = s2T[384:400]
    w2b = w2b.astype(E4)

    s3T = np.sign(W3).T.astype(np.float32)  # [200, 10]
    w3 = np.zeros((128, 2, DO), np.float32)
    w3[:, 0, :] = s3T[0:128]
    w3[0:72, 1, :] = s3T[128:200]
    w3 = w3.astype(E4)
    return w1ha, w1hb, w1lo, w1tl, w2a, w2b, w3


def _prep_x_core(xc):
    # xc: [8192, 784] fp32 -> feature-major hi/lo split
    xt = np.ascontiguousarray(xc.T.astype(np.float32))  # [784, 8192]
    hi = xt.astype(np.float16)
    lo = (xt - hi.astype(np.float32)).astype(np.float16)  # exact in fp16
    # hi k-tiles [16ch, 128, 6, 512]
    xhi = np.ascontiguousarray(
        hi[:768].reshape(KHI, 128, NCH, CH).transpose(2, 1, 0, 3)
    )
    # lo fp8 DR pairs [16ch, 128, 3, 2, 512]
    loq = (lo[:768].astype(np.float32) * LSC).astype(E4)
    xlo = np.ascontiguousarray(
        loq.reshape(KLO, 2, 128, NCH, CH).transpose(3, 2, 0, 1, 4)
    )
    # K tail rows 768:784 (hi + lo as fp16), replicated at strips 0/32/64
    xtl = np.empty((96, BL), np.float16)  # [96, 8192]
    for s in (0, 32, 64):
        xtl[s : s + 16] = hi[768:784]
        xtl[s + 16 : s + 32] = lo[768:784]
    xtl = np.ascontiguousarray(
        xtl.reshape(96, NCH, CH).transpose(1, 0, 2)
    )  # [16, 96, 512]
    return xhi, xlo, xtl


def kernel(x, W1, W2, W3, _trace=False, **_kw):
    nc = _build()
    w1ha, w1hb, w1lo, w1tl, w2a, w2b, w3 = _prep_weights(
        np.asarray(W1, np.float32), np.asarray(W2, np.float32), np.asarray(W3, np.float32)
    )
    x = np.asarray(x, np.float32).reshape(B, D0)

    in_maps = []
    for c in range(NCORES):
        xhi, xlo, xtl = _prep_x_core(x[c * BL : (c + 1) * BL])
        in_maps.append(
            {
                "xhi": xhi,
                "xlo": xlo,
                "xtl": xtl,
                "w1ha": w1ha,
                "w1hb": w1hb,
                "w1lo": w1lo,
                "w1tl": w1tl,
                "w2a": w2a,
                "w2b": w2b,
                "w3": w3,
            }
        )

    _ensure_axon_hooks()
    res = run_bass_kernel_spmd(nc, in_maps, core_ids=list(range(NCORES)), trace=_trace)

    out = np.empty((B, DO), np.float32)
    for c in range(NCORES):
        oc = res.results[c]["out"]  # [16, 10, 512]
        out[c * BL : (c + 1) * BL] = oc.transpose(0, 2, 1).reshape(BL, DO)
    if _trace:
        _cache["last_results"] = res
    return out


# revision 10
# speedup vs baseline: 1.0399x; 1.0399x over previous
"""Binarized 3-layer MLP on 8 TRN2 NeuronCores (data-parallel over batch).

Computation (matching the reference):
    h1  = x @ sign(W1).T          x: [65536, 784] fp32, W1: [400, 784]
    h2  = sign(h1) @ sign(W2).T   W2: [200, 400]
    out = sign(h2) @ sign(W3).T   W3: [10, 200]

Strategy (v2 — fp8 DoubleRow):
  - Batch sharded 8192 rows/core; weights replicated. Activations feature-major
    (features on SBUF partitions) so every contraction is already on partitions.
  - Layer 1 precision: x = hi + lo with hi = fp16(x), lo = fp16(x - hi) (exact).
    hi matmuls run in fp16 (K=784). The lo correction runs as fp8 DoubleRow:
    lo is quantized to e4m3 scaled by 2^12 and the weights carry sign(W1)*2^-12
    in e5m2 (exactly representable); one DR matmul contracts K=256. Total
    sign-flip error vs the fp32 reference measures rel=0.0071 on the actual
    inputs (gate is 2e-2) — dominated by the e4m3's 4-bit mantissa on lo,
    i.e. ~15 significand bits on x.
  - Layers 2/3 operate on exact +-1 values: e4m3 holds them exactly and fp32
    PSUM accumulation is exact, so layer 2 runs as fp8 DoubleRow (2 matmuls
    of K=256 instead of 4 of K=128) and layer 3 as plain fp8.
  - Layer-2 K layout: DR pairs are (partition p, half i). K-tile0 pairs
    h1 features (p | 128+p) = (m0 | m1) sign outputs; K-tile1 pairs
    (256+p | m4-packed strip). The m4 strip tile has sign outputs only at
    partitions 32jj:32jj+16 (chunk jj of the 4-chunk group, matching the
    col-strip-packed layer-1 m4 PSUM); weights for the other partitions are
    zero, and sign(memset-0 PSUM) = 0, so both operands vanish there.
  - The 400-row layer-1 output tiles as 128+128+128+16. The 16-row remainder
    (m4) is packed into one PSUM bank at partition strips 0/32/64/96 via
    tile_position col-tiling (4 chunks' matmuls run concurrently in distinct
    32-col PE groups). memset-to-zero + start=False keeps interleaved strip
    accumulation correct. Layer 3 (M=10) packs the same way.
  - K remainders (rows 768:784 of hi and lo) are folded into one 32-row fp16
    matmul per m-tile (lo is exact in fp16), replicated at partition strips
    0/32/64 so the three m-tiles' tail matmuls run concurrently.
"""

import contextlib
import ctypes
import os
import sys
import types

import numpy as np
import ml_dtypes

import concourse.bacc as bacc
import concourse.mybir as mybir
import concourse.tile as tile
from concourse.bass_utils import run_bass_kernel_spmd


def _ensure_axon_hooks():
    """concourse's trace path imports antenv.axon_hooks, which this image
    lacks; register a ctypes-backed stand-in so trace=True (or a stray
    BASS_TRACE=1 in the environment) cannot crash the run."""
    try:
        import antenv.axon_hooks  # noqa: F401
        return
    except ImportError:
        pass

    so_path = "/opt/axon/libaxon_pjrt.so"
    hook = None
    if os.path.exists(so_path):
        try:
            lib = ctypes.CDLL(so_path)
            if hasattr(lib, "axon_start_nrt_profile"):
                lib.axon_start_nrt_profile.argtypes = [
                    ctypes.POINTER(ctypes.c_int64),
                    ctypes.c_size_t,
                ]
                lib.axon_start_nrt_profile.restype = ctypes.c_int64
                lib.axon_stop_nrt_profile.argtypes = [ctypes.c_char_p]
                lib.axon_stop_nrt_profile.restype = ctypes.c_int64

                @contextlib.contextmanager
                def _hook(output_dir, device_ids):
                    import jax

                    jax.devices()
                    if device_ids:
                        ids = (ctypes.c_int64 * len(device_ids))(*device_ids)
                        rc = lib.axon_start_nrt_profile(ids, len(device_ids))
                    else:
                        rc = lib.axon_start_nrt_profile(None, 0)
                    if rc != 0:
                        raise RuntimeError(f"axon_start_nrt_profile rc={rc}")
                    try:
                        yield
                    finally:
                        lib.axon_stop_nrt_profile(str(output_dir).encode())

                hook = _hook
        except OSError:
            pass

    mod = types.ModuleType("antenv.axon_hooks")
    mod.get_axon_ntff_profile_hook = lambda: hook
    mod.set_axon_ntff_profile_hook = lambda h: None
    sys.modules["antenv.axon_hooks"] = mod

    import concourse.bass_utils as _bu

    _bu.upload_artifacts = lambda tmpdir: tmpdir

BF16 = np.dtype(ml_dtypes.bfloat16)
E4 = np.dtype(ml_dtypes.float8_e4m3)
E5 = np.dtype(ml_dtypes.float8_e5m2)

NCORES = 8
B = 65536
BL = B // NCORES          # 8192 rows per core
D0, H1, H2, DO = 784, 400, 200, 10
CH = 512                  # batch columns per chunk (PSUM bank = 512 fp32)
NCH = BL // CH            # 16 chunks per core
GRP = 4                   # chunks per packing group
KHI = 6                   # full 128-row fp16 k-tiles (rows 0:768)
KLO = 3                   # fp8 DoubleRow k-tiles of 256 (rows 0:768)
LSC = 2.0 ** 12           # lo scale: rhs carries lo*2^12, weights sign*2^-12
H2P = 208                 # padded layer-2 M so DR weight pair-stride % 16 == 0

_cache = {}


def _build():
    if "nc" in _cache:
        return _cache["nc"]

    f32 = mybir.dt.float32
    f16 = mybir.dt.float16
    f8e4 = mybir.dt.float8e4
    f8e5 = mybir.dt.float8e5
    Sign = mybir.ActivationFunctionType.Sign
    DR = mybir.MatmulPerfMode.DoubleRow

    nc = bacc.Bacc("TRN2", debug=False, num_devices=NCORES)

    d_xhi = nc.dram_tensor("xhi", [NCH, 128, KHI, CH], f16, kind="ExternalInput").ap()
    d_xlo = nc.dram_tensor("xlo", [NCH, 128, KLO, 2, CH], f8e4, kind="ExternalInput").ap()
    d_xtl = nc.dram_tensor("xtl", [NCH, 96, CH], f16, kind="ExternalInput").ap()
    # w1hi split so the first m-slab lands before the rest
    d_w1ha = nc.dram_tensor("w1ha", [128, KHI, 128], f16, kind="ExternalInput").ap()
    d_w1hb = nc.dram_tensor("w1hb", [128, KHI, H1 - 128], f16, kind="ExternalInput").ap()
    d_w1lo = nc.dram_tensor("w1lo", [128, KLO, 2, H1], f8e5, kind="ExternalInput").ap()
    d_w1tl = nc.dram_tensor("w1tl", [96, H1], f16, kind="ExternalInput").ap()
    d_w2a = nc.dram_tensor("w2a", [128, 2, H2P], f8e4, kind="ExternalInput").ap()
    d_w2b = nc.dram_tensor("w2b", [128, GRP, 2, H2P], f8e4, kind="ExternalInput").ap()
    d_w3 = nc.dram_tensor("w3", [128, 2, DO], f8e4, kind="ExternalInput").ap()
    d_out = nc.dram_tensor("out", [NCH, DO, CH], f32, kind="ExternalOutput").ap()

    with tile.TileContext(nc) as tc:
        with (
            tc.tile_pool(name="wp", bufs=1) as wp,
            tc.tile_pool(name="xp", bufs=6) as xp,
            tc.tile_pool(name="ap_", bufs=2) as apool,
            tc.tile_pool(name="a2p", bufs=2) as a2pool,
            tc.tile_pool(name="op", bufs=2) as op,
            tc.tile_pool(name="ps1p", bufs=1, space="PSUM") as ps1p,
            tc.tile_pool(name="ps2p", bufs=1, space="PSUM") as ps2p,
            tc.tile_pool(name="pspk", bufs=2, space="PSUM") as pspk,
        ):
            w1ha = wp.tile([128, KHI, 128], f16, name="w1ha")
            w1hb = wp.tile([128, KHI, H1 - 128], f16, name="w1hb")
            w1lo = wp.tile([128, KLO, 2, H1], f8e5, name="w1lo")
            w1tl = wp.tile([96, H1], f16, name="w1tl")
            w2a = wp.tile([128, 2, H2P], f8e4, name="w2a")
            w2b = wp.tile([128, GRP, 2, H2P], f8e4, name="w2b")
            w3sb = wp.tile([128, 2, DO], f8e4, name="w3sb")
            nc.sync.dma_start(out=w1ha[:], in_=d_w1ha)

            def w1h_slice(k, m_off, m_sz):
                if m_off == 0:
                    return w1ha[:, k, 0:m_sz]
                return w1hb[:, k, m_off - 128 : m_off - 128 + m_sz]

            def layer1_m123(xhi, xlo, xtl):
                """Full-width layer-1 m-tiles; returns the chunk's a1 tile
                [128, 4, CH] e4m3 with halves (m0 | m1 | m2 | m4-packed);
                the m4 half is written separately from ps4.

                A DoubleRow matmul in the MIDDLE of an accumulation group
                (acc_flags=0) costs 566ns vs 379 for start/stop ones, and
                adjacent DRs amortize the penalty — so each m-tile's 3 DR
                matmuls go at the HEAD of the group (first carries start),
                measured ~221ns/MM sustained vs ~403 when isolated."""
                a1 = apool.tile([128, 4, CH], f8e4, name="a1")
                pss = []
                for m in range(3):
                    ps = ps1p.tile(
                        [128, CH], f32, name=f"ps1_{m}", bufs=(2 if m == 0 else 1)
                    )
                    for t in range(KLO):
                        nc.tensor.matmul(
                            ps[:],
                            w1lo[:, t, :, m * 128 : m * 128 + 128],
                            xlo[:, t, :, :],
                            start=(t == 0),
                            stop=False,
                            perf_mode=DR,
                        )
                    for k in range(KHI):
                        nc.tensor.matmul(
                            ps[:],
                            w1h_slice(k, m * 128, 128),
                            xhi[:, k, :],
                            start=False,
                            stop=False,
                        )
                    pss.append(ps)
                # 32-row K tails (hi rows 768:784 + lo rows 768:784 as fp16),
                # replicated at partition strips 0/32/64 -> concurrent
                for m in range(3):
                    s = 32 * m
                    nc.tensor.matmul(
                        pss[m][:],
                        w1tl[s : s + 32, m * 128 : m * 128 + 128],
                        xtl[s : s + 32, :],
                        start=False,
                        stop=True,
                        tile_position=(s, 0),
                    )
                for m in range(3):
                    nc.scalar.activation(a1[:, m, :], pss[m][:], Sign)
                return a1

            def layer2(jj, a1, a2s):
                """Layer 2 for chunk jj (two DR matmuls per m-tile — both
                are start/stop flags, which run at full rate)."""
                for m in ((0, 1) if jj % 2 == 0 else (1, 0)):
                    sz = 128 if m == 0 else 72
                    ps = ps2p.tile([sz, CH], f32, name=f"ps2_{m}")
                    nc.tensor.matmul(
                        ps[:],
                        w2a[:, :, m * 128 : m * 128 + sz],
                        a1[:, 0:2, :],
                        start=True,
                        stop=False,
                        perf_mode=DR,
                    )
                    nc.tensor.matmul(
                        ps[:],
                        w2b[:, jj, :, m * 128 : m * 128 + sz],
                        a1[:, 2:4, :],
                        start=False,
                        stop=True,
                        perf_mode=DR,
                    )
                    at = a2pool.tile([sz, CH], f8e4, name=f"a2_{jj}_{m}")
                    nc.scalar.activation(at[:], ps[:], Sign)
                    a2s[jj][m] = at

            # HAM/P-state pre-warm: dummy matmuls on a scratch tile keep the
            # PE busy during the initial weight/x DMA wait so the first real
            # matmuls run at full clock (the activity window is ~3.4us).
            warm = wp.tile([128, 64], f16, name="warm")
            nc.vector.memset(warm[:], 1.0)
            wps = pspk.tile([64, 64], f32, name="wps", tag="pack")
            for _ in range(48):
                nc.tensor.matmul(wps[:], warm[:, 0:64], warm[:], start=True, stop=True)

            for g in range(NCH // GRP):
                xhis, xlos, xtls = [], [], []
                for jj in range(GRP):
                    c = g * GRP + jj
                    xhi = xp.tile([128, KHI, CH], f16, name="xhi")
                    xlo = xp.tile([128, KLO, 2, CH], f8e4, name="xlo")
                    xtl = xp.tile([96, CH], f16, name="xtl")
                    nc.sync.dma_start(out=xhi[:], in_=d_xhi[c])
                    nc.sync.dma_start(out=xlo[:], in_=d_xlo[c])
                    nc.sync.dma_start(out=xtl[:], in_=d_xtl[c])
                    xhis.append(xhi)
                    xlos.append(xlo)
                    xtls.append(xtl)
                    if g == 0 and jj == 0:
                        nc.sync.dma_start(out=w1hb[:], in_=d_w1hb)
                        nc.sync.dma_start(out=w1lo[:], in_=d_w1lo)
                        nc.sync.dma_start(out=w1tl[:], in_=d_w1tl)
                    if g == 0 and jj == 1:
                        nc.sync.dma_start(out=w2a[:], in_=d_w2a)
                        nc.sync.dma_start(out=w2b[:], in_=d_w2b)
                        nc.sync.dma_start(out=w3sb[:], in_=d_w3)

                # packed m4 PSUM bank: strips [32jj : 32jj+16] per chunk
                ps4 = pspk.tile([128, CH], f32, name="ps4", tag="pack")
                nc.vector.memset(ps4[:], 0.0)

                a1s = [None] * GRP
                a1s[0] = layer1_m123(xhis[0], xlos[0], xtls[0])
                a1s[1] = layer1_m123(xhis[1], xlos[1], xtls[1])

                # m4 packed: 4 col-tiled strips, interleaved for concurrency
                for k in range(KHI):
                    for jj in range(GRP):
                        s = 32 * jj
                        nc.tensor.matmul(
                            ps4[s : s + 16, :],
                            w1h_slice(k, 384, 16),
                            xhis[jj][:, k, :],
                            start=False,
                            stop=False,
                            tile_position=(0, s),
                        )
                for t in range(KLO):
                    for i in range(2):
                        for jj in range(GRP):
                            s = 32 * jj
                            nc.tensor.matmul(
                                ps4[s : s + 16, :],
                                w1lo[:, t, i, 384:400],
                                xlos[jj][:, t, i, :],
                                start=False,
                                stop=False,
                                tile_position=(0, s),
                            )
                for jj in range(GRP):
                    s = 32 * jj
                    nc.tensor.matmul(
                        ps4[s : s + 16, :],
                        w1tl[0:32, 384:400],
                        xtls[jj][0:32, :],
                        start=False,
                        stop=True,
                        tile_position=(0, s),
                    )
                # m4 sign into each chunk's a1[:, 3, :] (zeros elsewhere:
                # sign(0) = 0, and the paired w2b weights are 0 there too)
                nc.scalar.activation(a1s[0][:, 3, :], ps4[:], Sign)
                nc.scalar.activation(a1s[1][:, 3, :], ps4[:], Sign)

                # layer 3, packed into one PSUM bank at strips [32jj:32jj+10];
                # its plain-fp8 matmuls double as DR separators below
                a2s = [[None, None] for _ in range(GRP)]
                ps3 = pspk.tile([128, CH], f32, name="ps3", tag="pack")
                nc.vector.memset(ps3[:], 0.0)

                def l3(jj, k):
                    ks = 128 if k == 0 else 72
                    s = 32 * jj
                    nc.tensor.matmul(
                        ps3[s : s + DO, :],
                        w3sb[0:ks, k, :],
                        a2s[jj][k][0:ks, :],
                        start=False,
                        stop=(k == 1),
                        tile_position=(0, s),
                    )

                layer2(0, a1s[0], a2s)
                layer2(1, a1s[1], a2s)
                a1s[2] = layer1_m123(xhis[2], xlos[2], xtls[2])
                nc.scalar.activation(a1s[2][:, 3, :], ps4[:], Sign)
                layer2(2, a1s[2], a2s)
                a1s[3] = layer1_m123(xhis[3], xlos[3], xtls[3])
                nc.scalar.activation(a1s[3][:, 3, :], ps4[:], Sign)
                layer2(3, a1s[3], a2s)
                for k in range(2):
                    for jj in range(GRP):
                        l3(jj, k)
                osb = op.tile([128, CH], f32, name="osb")
                nc.vector.tensor_copy(osb[:], ps3[:])
                for jj in range(GRP):
                    s = 32 * jj
                    nc.sync.dma_start(
                        out=d_out[g * GRP + jj], in_=osb[s : s + DO, :]
                    )

    nc.compile()
    _cache["nc"] = nc
    return nc


def _prep_weights(W1, W2, W3):
    s1T = np.sign(W1).T.astype(np.float32)  # [784, 400]
    # hi weights: rows 0:768 as 6 k-tiles of 128
    w1h = np.ascontiguousarray(
        s1T[:768].reshape(KHI, 128, H1).transpose(1, 0, 2)
    ).astype(np.float16)  # [128, 6, 400]
    w1ha = np.ascontiguousarray(w1h[:, :, 0:128])
    w1hb = np.ascontiguousarray(w1h[:, :, 128:H1])
    # lo weights: rows 0:768 as 3 DR k-tiles of (2 x 128), scaled 2^-12 (e5m2)
    w1lo = np.ascontiguousarray(
        (s1T[:768] / LSC).reshape(KLO, 2, 128, H1).transpose(2, 0, 1, 3)
    ).astype(E5)  # [128, 3, 2, 400]
    # K tail (rows 768:784): strips 0/32/64, each [hi-tail | lo-tail] with
    # identical +-1 weights (the rhs carries hi and lo values separately)
    w1tl = np.zeros((96, H1), np.float32)
    for s in (0, 32, 64):
        w1tl[s : s + 16] = s1T[768:784]
        w1tl[s + 16 : s + 32] = s1T[768:784]
    w1tl = w1tl.astype(np.float16)

    s2T = np.sign(W2).T.astype(np.float32)  # [400, 200]
    w2a = np.zeros((128, 2, H2P), np.float32)
    w2a[:, 0, :H2] = s2T[0:128]
    w2a[:, 1, :H2] = s2T[128:256]
    w2a = w2a.astype(E4)
    w2b = np.zeros((128, GRP, 2, H2P), np.float32)
    for jj in range(GRP):
        w2b[:, jj, 0, :H2] = s2T[256:384]
        w2b[32 * jj : 32 * jj + 16, jj, 1, :H2] = s2T[384:400]
    w2b = w2b.astype(E4)

    s3T = np.sign(W3).T.astype(np.float32)  # [200, 10]
    w3 = np.zeros((128, 2, DO), np.float32)
    w3[:, 0, :] = s3T[0:128]
    w3[0:72, 1, :] = s3T[128:200]
    w3 = w3.astype(E4)
    return w1ha, w1hb, w1lo, w1tl, w2a, w2b, w3


def _prep_x_core(xc):
    # xc: [8192, 784] fp32 -> feature-major hi/lo split
    xt = np.ascontiguousarray(xc.T.astype(np.float32))  # [784, 8192]
    hi = xt.astype(np.float16)
    lo = (xt - hi.astype(np.float32)).astype(np.float16)  # exact in fp16
    # hi k-tiles [16ch, 128, 6, 512]
    xhi = np.ascontiguousarray(
        hi[:768].reshape(KHI, 128, NCH, CH).transpose(2, 1, 0, 3)
    )
    # lo fp8 DR pairs [16ch, 128, 3, 2, 512]
    loq = (lo[:768].astype(np.float32) * LSC).astype(E4)
    xlo = np.ascontiguousarray(
        loq.reshape(KLO, 2, 128, NCH, CH).transpose(3, 2, 0, 1, 4)
    )
    # K tail rows 768:784 (hi + lo as fp16), replicated at strips 0/32/64
    xtl = np.empty((96, BL), np.float16)  # [96, 8192]
    for s in (0, 32, 64):
        xtl[s : s + 16] = hi[768:784]
        xtl[s + 16 : s + 32] = lo[768:784]
    xtl = np.ascontiguousarray(
        xtl.reshape(96, NCH, CH).transpose(1, 0, 2)
    )  # [16, 96, 512]
    return xhi, xlo, xtl


def kernel(x, W1, W2, W3, _trace=False, **_kw):
    nc = _build()
    w1ha, w1hb, w1lo, w1tl, w2a, w2b, w3 = _prep_weights(
        np.asarray(W1, np.float32), np.asarray(W2, np.float32), np.asarray(W3, np.float32)
    )
    x = np.asarray(x, np.float32).reshape(B, D0)

    in_maps = []
    for c in range(NCORES):
        xhi, xlo, xtl = _prep_x_core(x[c * BL : (c + 1) * BL])
        in_maps.append(
            {
                "xhi": xhi,
                "xlo": xlo,
                "xtl": xtl,
                "w1ha": w1ha,
                "w1hb": w1hb,
                "w1lo": w1lo,
                "w1tl": w1tl,
                "w2a": w2a,
                "w2b": w2b,
                "w3": w3,
            }
        )

    _ensure_axon_hooks()
    res = run_bass_kernel_spmd(nc, in_maps, core_ids=list(range(NCORES)), trace=_trace)

    out = np.empty((B, DO), np.float32)
    for c in range(NCORES):
        oc = res.results[c]["out"]  # [16, 10, 512]
        out[c * BL : (c + 1) * BL] = oc.transpose(0, 2, 1).reshape(BL, DO)
    if _trace:
        _cache["last_results"] = res
    return out


# revision 13
# speedup vs baseline: 1.0726x; 1.0314x over previous
"""Binarized 3-layer MLP on 8 TRN2 NeuronCores (data-parallel over batch).

Computation (matching the reference):
    h1  = x @ sign(W1).T          x: [65536, 784] fp32, W1: [400, 784]
    h2  = sign(h1) @ sign(W2).T   W2: [200, 400]
    out = sign(h2) @ sign(W3).T   W3: [10, 200]

Strategy (v2 — fp8 DoubleRow):
  - Batch sharded 8192 rows/core; weights replicated. Activations feature-major
    (features on SBUF partitions) so every contraction is already on partitions.
  - Layer 1 precision: x = hi + lo with hi = fp16(x), lo = fp16(x - hi) (exact).
    hi matmuls run in fp16 (K=784). The lo correction runs as fp8 DoubleRow:
    lo is quantized to e4m3 scaled by 2^12 and the weights carry sign(W1)*2^-12
    in e5m2 (exactly representable); one DR matmul contracts K=256. Total
    sign-flip error vs the fp32 reference measures rel=0.0071 on the actual
    inputs (gate is 2e-2) — dominated by the e4m3's 4-bit mantissa on lo,
    i.e. ~15 significand bits on x.
  - Layers 2/3 operate on exact +-1 values: e4m3 holds them exactly and fp32
    PSUM accumulation is exact, so layer 2 runs as fp8 DoubleRow (2 matmuls
    of K=256 instead of 4 of K=128) and layer 3 as plain fp8.
  - Layer-2 K layout: DR pairs are (partition p, half i). K-tile0 pairs
    h1 features (p | 128+p) = (m0 | m1) sign outputs; K-tile1 pairs
    (256+p | m4-packed strip). The m4 strip tile has sign outputs only at
    partitions 32jj:32jj+16 (chunk jj of the 4-chunk group, matching the
    col-strip-packed layer-1 m4 PSUM); weights for the other partitions are
    zero, and sign(memset-0 PSUM) = 0, so both operands vanish there.
  - The 400-row layer-1 output tiles as 128+128+128+16. The 16-row remainder
    (m4) is packed into one PSUM bank at partition strips 0/32/64/96 via
    tile_position col-tiling (4 chunks' matmuls run concurrently in distinct
    32-col PE groups). memset-to-zero + start=False keeps interleaved strip
    accumulation correct. Layer 3 (M=10) packs the same way.
  - K remainders (rows 768:784 of hi and lo) are folded into one 32-row fp16
    matmul per m-tile (lo is exact in fp16), replicated at partition strips
    0/32/64 so the three m-tiles' tail matmuls run concurrently.
"""

import contextlib
import ctypes
import os
import sys
import types

import numpy as np
import ml_dtypes

import concourse.bacc as bacc
import concourse.mybir as mybir
import concourse.tile as tile
from concourse.bass_utils import run_bass_kernel_spmd


def _ensure_axon_hooks():
    """concourse's trace path imports antenv.axon_hooks, which this image
    lacks; register a ctypes-backed stand-in so trace=True (or a stray
    BASS_TRACE=1 in the environment) cannot crash the run."""
    try:
        import antenv.axon_hooks  # noqa: F401
        return
    except ImportError:
        pass

    so_path = "/opt/axon/libaxon_pjrt.so"
    hook = None
    if os.path.exists(so_path):
        try:
            lib = ctypes.CDLL(so_path)
            if hasattr(lib, "axon_start_nrt_profile"):
                lib.axon_start_nrt_profile.argtypes = [
                    ctypes.POINTER(ctypes.c_int64),
                    ctypes.c_size_t,
                ]
                lib.axon_start_nrt_profile.restype = ctypes.c_int64
                lib.axon_stop_nrt_profile.argtypes = [ctypes.c_char_p]
                lib.axon_stop_nrt_profile.restype = ctypes.c_int64

                @contextlib.contextmanager
                def _hook(output_dir, device_ids):
                    import jax

                    jax.devices()
                    if device_ids:
                        ids = (ctypes.c_int64 * len(device_ids))(*device_ids)
                        rc = lib.axon_start_nrt_profile(ids, len(device_ids))
                    else:
                        rc = lib.axon_start_nrt_profile(None, 0)
                    if rc != 0:
                        raise RuntimeError(f"axon_start_nrt_profile rc={rc}")
                    try:
                        yield
                    finally:
                        lib.axon_stop_nrt_profile(str(output_dir).encode())

                hook = _hook
        except OSError:
            pass

    mod = types.ModuleType("antenv.axon_hooks")
    mod.get_axon_ntff_profile_hook = lambda: hook
    mod.set_axon_ntff_profile_hook = lambda h: None
    sys.modules["antenv.axon_hooks"] = mod

    import concourse.bass_utils as _bu

    _bu.upload_artifacts = lambda tmpdir: tmpdir

BF16 = np.dtype(ml_dtypes.bfloat16)
E4 = np.dtype(ml_dtypes.float8_e4m3)
E5 = np.dtype(ml_dtypes.float8_e5m2)

NCORES = 8
B = 65536
BL = B // NCORES          # 8192 rows per core
D0, H1, H2, DO = 784, 400, 200, 10
CH = 512                  # batch columns per chunk (PSUM bank = 512 fp32)
NCH = BL // CH            # 16 chunks per core
GRP = 4                   # chunks per packing group
KHI = 6                   # full 128-row fp16 k-tiles (rows 0:768)
KLO = 3                   # fp8 DoubleRow k-tiles of 256 (rows 0:768)
LSC = 2.0 ** 12           # lo scale: rhs carries lo*2^12, weights sign*2^-12
H2P = 208                 # padded layer-2 M so DR weight pair-stride % 16 == 0

_cache = {}


def _build():
    if "nc" in _cache:
        return _cache["nc"]

    f32 = mybir.dt.float32
    f16 = mybir.dt.float16
    f8e4 = mybir.dt.float8e4
    f8e5 = mybir.dt.float8e5
    Sign = mybir.ActivationFunctionType.Sign
    DR = mybir.MatmulPerfMode.DoubleRow

    nc = bacc.Bacc("TRN2", debug=False, num_devices=NCORES)

    d_xhi = nc.dram_tensor("xhi", [NCH, 128, KHI, CH], f16, kind="ExternalInput").ap()
    d_xlo = nc.dram_tensor("xlo", [NCH, 128, KLO, 2, CH], f8e4, kind="ExternalInput").ap()
    d_xtl = nc.dram_tensor("xtl", [NCH, 96, CH], f16, kind="ExternalInput").ap()
    # w1hi split so the first m-slab lands before the rest
    d_w1ha = nc.dram_tensor("w1ha", [128, KHI, 128], f16, kind="ExternalInput").ap()
    d_w1hb = nc.dram_tensor("w1hb", [128, KHI, H1 - 128], f16, kind="ExternalInput").ap()
    d_w1lo = nc.dram_tensor("w1lo", [128, KLO, 2, H1], f8e5, kind="ExternalInput").ap()
    d_w1tl = nc.dram_tensor("w1tl", [96, H1], f16, kind="ExternalInput").ap()
    d_w2a = nc.dram_tensor("w2a", [128, 2, H2P], f8e4, kind="ExternalInput").ap()
    d_w2b = nc.dram_tensor("w2b", [128, GRP, 2, H2P], f8e4, kind="ExternalInput").ap()
    d_w3 = nc.dram_tensor("w3", [128, 2, DO], f8e4, kind="ExternalInput").ap()
    d_out = nc.dram_tensor("out", [NCH, DO, CH], f32, kind="ExternalOutput").ap()

    with tile.TileContext(nc) as tc:
        with (
            tc.tile_pool(name="wp", bufs=1) as wp,
            tc.tile_pool(name="xp", bufs=6) as xp,
            tc.tile_pool(name="ap_", bufs=2) as apool,
            tc.tile_pool(name="a2p", bufs=2) as a2pool,
            tc.tile_pool(name="op", bufs=2) as op,
            tc.tile_pool(name="ps1p", bufs=1, space="PSUM") as ps1p,
            tc.tile_pool(name="ps2p", bufs=1, space="PSUM") as ps2p,
            tc.tile_pool(name="pspk", bufs=2, space="PSUM") as pspk,
        ):
            w1ha = wp.tile([128, KHI, 128], f16, name="w1ha")
            w1hb = wp.tile([128, KHI, H1 - 128], f16, name="w1hb")
            w1lo = wp.tile([128, KLO, 2, H1], f8e5, name="w1lo")
            w1tl = wp.tile([96, H1], f16, name="w1tl")
            w2a = wp.tile([128, 2, H2P], f8e4, name="w2a")
            w2b = wp.tile([128, GRP, 2, H2P], f8e4, name="w2b")
            w3sb = wp.tile([128, 2, DO], f8e4, name="w3sb")
            nc.sync.dma_start(out=w1ha[:], in_=d_w1ha)

            def w1h_slice(k, m_off, m_sz):
                if m_off == 0:
                    return w1ha[:, k, 0:m_sz]
                return w1hb[:, k, m_off - 128 : m_off - 128 + m_sz]

            def layer1_m123(xhi, xlo, xtl):
                """Full-width layer-1 m-tiles; returns the chunk's a1 tile
                [128, 4, CH] e4m3 with halves (m0 | m1 | m2 | m4-packed);
                the m4 half is written separately from ps4.

                A DoubleRow matmul in the MIDDLE of an accumulation group
                (acc_flags=0) costs 566ns vs 379 for start/stop ones, and
                adjacent DRs amortize the penalty — so each m-tile's 3 DR
                matmuls go at the HEAD of the group (first carries start),
                measured ~221ns/MM sustained vs ~403 when isolated."""
                a1 = apool.tile([128, 4, CH], f8e4, name="a1")
                pss = [
                    ps1p.tile([128, CH], f32, name=f"ps1_{m}", bufs=(2 if m == 0 else 1))
                    for m in range(3)
                ]
                # single 9-DR run, t-outer/m-inner so each bank's start
                # (flags=1) is adjacent to another start: only the one
                # (start->middle) boundary in the run pays the DR stall
                for t in range(KLO):
                    for m in range(3):
                        nc.tensor.matmul(
                            pss[m][:],
                            w1lo[:, t, :, m * 128 : m * 128 + 128],
                            xlo[:, t, :, :],
                            start=(t == 0),
                            stop=False,
                            perf_mode=DR,
                        )
                for m in range(3):
                    for k in range(KHI):
                        nc.tensor.matmul(
                            pss[m][:],
                            w1h_slice(k, m * 128, 128),
                            xhi[:, k, :],
                            start=False,
                            stop=False,
                        )
                # 32-row K tails (hi rows 768:784 + lo rows 768:784 as fp16),
                # replicated at partition strips 0/32/64 -> concurrent
                for m in range(3):
                    s = 32 * m
                    nc.tensor.matmul(
                        pss[m][:],
                        w1tl[s : s + 32, m * 128 : m * 128 + 128],
                        xtl[s : s + 32, :],
                        start=False,
                        stop=True,
                        tile_position=(s, 0),
                    )
                for m in range(3):
                    nc.scalar.activation(a1[:, m, :], pss[m][:], Sign)
                return a1

            def layer2(jj, a1, a2s):
                """Layer 2 for chunk jj (two DR matmuls per m-tile — both
                are start/stop flags, which run at full rate)."""
                for m in ((0, 1) if jj % 2 == 0 else (1, 0)):
                    sz = 128 if m == 0 else 72
                    ps = ps2p.tile([sz, CH], f32, name=f"ps2_{m}")
                    nc.tensor.matmul(
                        ps[:],
                        w2a[:, :, m * 128 : m * 128 + sz],
                        a1[:, 0:2, :],
                        start=True,
                        stop=False,
                        perf_mode=DR,
                    )
                    nc.tensor.matmul(
                        ps[:],
                        w2b[:, jj, :, m * 128 : m * 128 + sz],
                        a1[:, 2:4, :],
                        start=False,
                        stop=True,
                        perf_mode=DR,
                    )
                    at = a2pool.tile([sz, CH], f8e4, name=f"a2_{jj}_{m}")
                    nc.scalar.activation(at[:], ps[:], Sign)
                    a2s[jj][m] = at

            # HAM/P-state pre-warm: dummy matmuls on a scratch tile keep the
            # PE busy during the initial weight/x DMA wait so the first real
            # matmuls run at full clock (the activity window is ~3.4us).
            warm = wp.tile([128, 64], f16, name="warm")
            nc.vector.memset(warm[:], 1.0)
            wps = pspk.tile([64, 64], f32, name="wps", tag="pack")
            for _ in range(96):
                nc.tensor.matmul(wps[:], warm[:, 0:64], warm[:], start=True, stop=True)

            for g in range(NCH // GRP):
                xhis, xlos, xtls = [], [], []
                for jj in range(GRP):
                    c = g * GRP + jj
                    xhi = xp.tile([128, KHI, CH], f16, name="xhi")
                    xlo = xp.tile([128, KLO, 2, CH], f8e4, name="xlo")
                    xtl = xp.tile([96, CH], f16, name="xtl")
                    # xlo first: the chunk's PE stream begins with the DR run
                    nc.sync.dma_start(out=xlo[:], in_=d_xlo[c])
                    if g == 0 and jj == 0:
                        nc.sync.dma_start(out=w1lo[:], in_=d_w1lo)
                    nc.sync.dma_start(out=xhi[:], in_=d_xhi[c])
                    nc.sync.dma_start(out=xtl[:], in_=d_xtl[c])
                    xhis.append(xhi)
                    xlos.append(xlo)
                    xtls.append(xtl)
                    if g == 0 and jj == 0:
                        nc.sync.dma_start(out=w1hb[:], in_=d_w1hb)
                        nc.sync.dma_start(out=w1tl[:], in_=d_w1tl)
                    if g == 0 and jj == 1:
                        nc.sync.dma_start(out=w2a[:], in_=d_w2a)
                        nc.sync.dma_start(out=w2b[:], in_=d_w2b)
                        nc.sync.dma_start(out=w3sb[:], in_=d_w3)

                # packed m4 PSUM bank: strips [32jj : 32jj+16] per chunk
                ps4 = pspk.tile([128, CH], f32, name="ps4", tag="pack")
                nc.vector.memset(ps4[:], 0.0)

                a1s = [None] * GRP
                a1s[0] = layer1_m123(xhis[0], xlos[0], xtls[0])
                a1s[1] = layer1_m123(xhis[1], xlos[1], xtls[1])

                # m4 packed: 4 col-tiled strips, interleaved for concurrency
                for k in range(KHI):
                    for jj in range(GRP):
                        s = 32 * jj
                        nc.tensor.matmul(
                            ps4[s : s + 16, :],
                            w1h_slice(k, 384, 16),
                            xhis[jj][:, k, :],
                            start=False,
                            stop=False,
                            tile_position=(0, s),
                        )
                for t in range(KLO):
                    for i in range(2):
                        for jj in range(GRP):
                            s = 32 * jj
                            nc.tensor.matmul(
                                ps4[s : s + 16, :],
                                w1lo[:, t, i, 384:400],
                                xlos[jj][:, t, i, :],
                                start=False,
                                stop=False,
                                tile_position=(0, s),
                            )
                for jj in range(GRP):
                    s = 32 * jj
                    nc.tensor.matmul(
                        ps4[s : s + 16, :],
                        w1tl[0:32, 384:400],
                        xtls[jj][0:32, :],
                        start=False,
                        stop=True,
                        tile_position=(0, s),
                    )
                # m4 sign into each chunk's a1[:, 3, :] (zeros elsewhere:
                # sign(0) = 0, and the paired w2b weights are 0 there too)
                nc.scalar.activation(a1s[0][:, 3, :], ps4[:], Sign)
                nc.scalar.activation(a1s[1][:, 3, :], ps4[:], Sign)

                # layer 3, packed into one PSUM bank at strips [32jj:32jj+10];
                # its plain-fp8 matmuls double as DR separators below
                a2s = [[None, None] for _ in range(GRP)]
                ps3 = pspk.tile([128, CH], f32, name="ps3", tag="pack")
                nc.vector.memset(ps3[:], 0.0)

                def l3(jj, k):
                    ks = 128 if k == 0 else 72
                    s = 32 * jj
                    nc.tensor.matmul(
                        ps3[s : s + DO, :],
                        w3sb[0:ks, k, :],
                        a2s[jj][k][0:ks, :],
                        start=False,
                        stop=(k == 1),
                        tile_position=(0, s),
                    )

                layer2(0, a1s[0], a2s)
                layer2(1, a1s[1], a2s)
                a1s[2] = layer1_m123(xhis[2], xlos[2], xtls[2])
                nc.scalar.activation(a1s[2][:, 3, :], ps4[:], Sign)
                layer2(2, a1s[2], a2s)
                a1s[3] = layer1_m123(xhis[3], xlos[3], xtls[3])
                nc.scalar.activation(a1s[3][:, 3, :], ps4[:], Sign)
                layer2(3, a1s[3], a2s)
                for k in range(2):
                    for jj in range(GRP):
                        l3(jj, k)
                osb = op.tile([128, CH], f32, name="osb")
                nc.vector.tensor_copy(osb[:], ps3[:])
                for jj in range(GRP):
                    s = 32 * jj
                    nc.sync.dma_start(
                        out=d_out[g * GRP + jj], in_=osb[s : s + DO, :]
                    )

    nc.compile()
    _cache["nc"] = nc
    return nc


def _prep_weights(W1, W2, W3):
    s1T = np.sign(W1).T.astype(np.float32)  # [784, 400]
    # hi weights: rows 0:768 as 6 k-tiles of 128
    w1h = np.ascontiguousarray(
        s1T[:768].reshape(KHI, 128, H1).transpose(1, 0, 2)
    ).astype(np.float16)  # [128, 6, 400]
    w1ha = np.ascontiguousarray(w1h[:, :, 0:128])
    w1hb = np.ascontiguousarray(w1h[:, :, 128:H1])
    # lo weights: rows 0:768 as 3 DR k-tiles of (2 x 128), scaled 2^-12 (e5m2)
    w1lo = np.ascontiguousarray(
        (s1T[:768] / LSC).reshape(KLO, 2, 128, H1).transpose(2, 0, 1, 3)
    ).astype(E5)  # [128, 3, 2, 400]
    # K tail (rows 768:784): strips 0/32/64, each [hi-tail | lo-tail] with
    # identical +-1 weights (the rhs carries hi and lo values separately)
    w1tl = np.zeros((96, H1), np.float32)
    for s in (0, 32, 64):
        w1tl[s : s + 16] = s1T[768:784]
        w1tl[s + 16 : s + 32] = s1T[768:784]
    w1tl = w1tl.astype(np.float16)

    s2T = np.sign(W2).T.astype(np.float32)  # [400, 200]
    w2a = np.zeros((128, 2, H2P), np.float32)
    w2a[:, 0, :H2] = s2T[0:128]
    w2a[:, 1, :H2] = s2T[128:256]
    w2a = w2a.astype(E4)
    w2b = np.zeros((128, GRP, 2, H2P), np.float32)
    for jj in range(GRP):
        w2b[:, jj, 0, :H2] = s2T[256:384]
        w2b[32 * jj : 32 * jj + 16, jj, 1, :H2] = s2T[384:400]
    w2b = w2b.astype(E4)

    s3T = np.sign(W3).T.astype(np.float32)  # [200, 10]
    w3 = np.zeros((128, 2, DO), np.float32)
    w3[:, 0, :] = s3T[0:128]
    w3[0:72, 1, :] = s3T[128:200]
    w3 = w3.astype(E4)
    return w1ha, w1hb, w1lo, w1tl, w2a, w2b, w3


def _prep_x_core(xc):
    # xc: [8192, 784] fp32 -> feature-major hi/lo split
    xt = np.ascontiguousarray(xc.T.astype(np.float32))  # [784, 8192]
    hi = xt.astype(np.float16)
    lo = (xt - hi.astype(np.float32)).astype(np.float16)  # exact in fp16
    # hi k-tiles [16ch, 128, 6, 512]
    xhi = np.ascontiguousarray(
        hi[:768].reshape(KHI, 128, NCH, CH).transpose(2, 1, 0, 3)
    )
    # lo fp8 DR pairs [16ch, 128, 3, 2, 512]
    loq = (lo[:768].astype(np.float32) * LSC).astype(E4)
    xlo = np.ascontiguousarray(
        loq.reshape(KLO, 2, 128, NCH, CH).transpose(3, 2, 0, 1, 4)
    )
    # K tail rows 768:784 (hi + lo as fp16), replicated at strips 0/32/64
    xtl = np.empty((96, BL), np.float16)  # [96, 8192]
    for s in (0, 32, 64):
        xtl[s : s + 16] = hi[768:784]
        xtl[s + 16 : s + 32] = lo[768:784]
    xtl = np.ascontiguousarray(
        xtl.reshape(96, NCH, CH).transpose(1, 0, 2)
    )  # [16, 96, 512]
    return xhi, xlo, xtl


def kernel(x, W1, W2, W3, _trace=False, **_kw):
    nc = _build()
    w1ha, w1hb, w1lo, w1tl, w2a, w2b, w3 = _prep_weights(
        np.asarray(W1, np.float32), np.asarray(W2, np.float32), np.asarray(W3, np.float32)
    )
    x = np.asarray(x, np.float32).reshape(B, D0)

    in_maps = []
    for c in range(NCORES):
        xhi, xlo, xtl = _prep_x_core(x[c * BL : (c + 1) * BL])
        in_maps.append(
            {
                "xhi": xhi,
                "xlo": xlo,
                "xtl": xtl,
                "w1ha": w1ha,
                "w1hb": w1hb,
                "w1lo": w1lo,
                "w1tl": w1tl,
                "w2a": w2a,
                "w2b": w2b,
                "w3": w3,
            }
        )

    _ensure_axon_hooks()
    res = run_bass_kernel_spmd(nc, in_maps, core_ids=list(range(NCORES)), trace=_trace)

    out = np.empty((B, DO), np.float32)
    for c in range(NCORES):
        oc = res.results[c]["out"]  # [16, 10, 512]
        out[c * BL : (c + 1) * BL] = oc.transpose(0, 2, 1).reshape(BL, DO)
    if _trace:
        _cache["last_results"] = res
    return out
